# revision 32
# baseline (speedup 1.0000x reference)
"""Trainium2 Bass kernel for nn_AE_rnn (bi-LSTM autoencoder over vocab logits).

Strategy (8 NeuronCores, SPMD, no collectives):
- logits GEMM (2048x26 @ 26x32000, 262MB out) sharded over vocab: each core
  computes a (4000, 2048) transposed logits shard via f32r matmuls with the
  bias folded in as an extra contraction row, PSUM -> SBUF -> DRAM.
- rows 1..2047 of dec_h come from the zero-state decoder path (elementwise,
  cheap, available immediately) so the big GEMM starts at t=0 of the kernel.
- row 0 needs the encoder. The 4 bidirectional LSTM scans are replaced by a
  truncated-window (W=128, forget-gate decay) joint fixed-point iteration:
  gates from the previous h estimate, cell state via the DVE
  tensor_tensor_scan linear-recurrence instruction, 8 iterations to fp32
  accuracy. Chains are packed as columns; sigmoid via tanh half-angle so the
  scalar engine only ever computes tanh. Encoder replicated on every core
  (no collectives); each core computes logits row 0 for its own vocab shard.
- embeddings output gathered row-sharded (256 rows/core) via indirect DMA.

Hardware layout constraints honored here:
- engine SBUF access patterns must start at partition 0/32/64/96, and all
  SBUF operands of one vector op must share the same start partition. So the
  13-row gate groups are padded into 32-row blocks (gate tiles: 128 rows
  [i|f|o|g], fwd at +0:13, bwd at +16:29 of each block, pads stay zero), the
  tanh of each gate block gets its own 32-row tile (PSUM input slices are
  exempt from the same-start rule), and the cell update works on the doubled
  state c2=2c:  z2 = (Ti+1)*Tg,  c2 = scan(sf, z2),  h = 0.5*(To+1)*tanh(c2/2)
  so every vector op sees operands based at partition 0.

kernel(**inputs) takes the full unsharded inputs and returns (logits, embs).
"""
from contextlib import ExitStack

import numpy as np

import concourse.bass as bass
import concourse.tile as tile
from concourse import bacc, mybir
from concourse import bass_utils
from concourse.bass_interp import get_hw_module
from concourse.masks import make_identity

F32 = mybir.dt.float32
F32R = mybir.dt.float32r
I32 = mybir.dt.int32
AF = mybir.ActivationFunctionType
OP = mybir.AluOpType

L = 2048
H = 13
W = 128            # live window per chain
NITER = 5
NCORE = 8
VSH = 32000 // NCORE       # 4000 vocab rows per core
ESH = L // NCORE           # 256 embs rows per core
CH = 512                   # t-chunk for dec_h / GEMM

GB = {"i": 0, "f": 32, "o": 64, "g": 96}   # gate block starts (128 rows)
TORCH = {"i": 0, "f": 13, "g": 26, "o": 39}


def _r(t):
    return t[:].bitcast(F32R)


def _pack_dir(p_f, p_b):
    """Pack one (fwd,bwd) LSTM cell pair into the padded block layout.
    Returns wih (128, in_sz), whh (32, 128), bih (128,), bhh (128,)."""
    in_sz = np.asarray(p_f[0]).shape[1]
    wih = np.zeros((128, in_sz), np.float32)
    whh = np.zeros((32, 128), np.float32)
    bih_p = np.zeros(128, np.float32)
    bhh_p = np.zeros(128, np.float32)
    for d, p in ((0, p_f), (1, p_b)):
        Wih, Whh, bih, bhh = [np.asarray(a, dtype=np.float32) for a in p]
        for gt in "ifog":
            src = slice(TORCH[gt], TORCH[gt] + H)
            dst = slice(GB[gt] + 16 * d, GB[gt] + 16 * d + H)
            wih[dst] = Wih[src]
            whh[16 * d:16 * d + H, dst] = Whh[src].T
            bih_p[dst] = bih[src]
            bhh_p[dst] = bhh[src]
    return wih, whh, bih_p, bhh_p


def _dir_col_mask(d):
    m = np.zeros(128, np.float32)
    for gt in "ifog":
        m[GB[gt] + 16 * d:GB[gt] + 16 * d + H] = 1.0
    return m


def _group_cols(vec128):
    """(128,) packed vector -> (32, 4) one column per gate group [i,f,o,g]."""
    return np.stack([vec128[GB[gt]:GB[gt] + 32] for gt in "ifog"], 1)


def _emit(ctx: ExitStack, tc: tile.TileContext, A: dict):
    nc = tc.nc
    sb = ctx.enter_context(tc.tile_pool(name="sb", bufs=1))
    sb_g = ctx.enter_context(tc.tile_pool(name="sb_g", bufs=2))
    sb_l = ctx.enter_context(tc.tile_pool(name="sb_l", bufs=3))
    sb_e = ctx.enter_context(tc.tile_pool(name="sb_e", bufs=2))
    sb_t = ctx.enter_context(tc.tile_pool(name="sb_t", bufs=2))
    ps_tr = ctx.enter_context(tc.tile_pool(name="ps_tr", bufs=1, space="PSUM"))
    ps_u = ctx.enter_context(tc.tile_pool(name="ps_u", bufs=1, space="PSUM"))
    ps_g = ctx.enter_context(tc.tile_pool(name="ps_g", bufs=1, space="PSUM"))
    ps_dz = ctx.enter_context(tc.tile_pool(name="ps_dz", bufs=1, space="PSUM"))
    ps_ms = ps_dz
    ps_mm = ctx.enter_context(tc.tile_pool(name="ps_mm", bufs=4, space="PSUM"))

    # ---------------- constants + weight loads ----------------
    identf = sb.tile([128, 128], F32, tag="identf")
    make_identity(nc, identf[:])
    ident = sb.tile([128, 128], F32R, tag="ident")
    nc.vector.tensor_copy(ident[:], identf[:])
    half4 = sb.tile([32, 4], F32, tag="half4")
    nc.gpsimd.memset(half4[:], 1.0)
    nc.gpsimd.memset(half4[:, 0:3], 0.5)     # i,f,o halved; g not

    def load(name, shape, dtype=F32, eng=None):
        t = sb.tile(list(shape), dtype, tag=name)
        (eng or nc.sync).dma_start(t[:], A[name][:])
        return t

    # dec-zero / early-GEMM critical loads first on the sync queue
    x_sb = sb.tile([1, L], I32, tag="x_sb")
    nc.sync.dma_start(x_sb[:], A["x"][:])
    wd_r = load("wd_r", (1, 128))
    bdg = load("bdg", (32, 8))
    wd_g = load("wd_g", (32, 4))
    wb = load("wb", (33, VSH), F32R)
    xw = sb.tile([128, 4], I32, tag="xw")
    nc.sync.dma_start(xw[:], A["x_win"].rearrange("(c p) -> p c", p=128))
    xmy = sb.tile([128, 2], I32, tag="xmy")
    nc.sync.dma_start(xmy[:], A["x_my"].rearrange("(c p) -> p c", p=128))
    s_f = sb.tile([1, L], F32, tag="s_f")
    nc.vector.tensor_copy(s_f[:], x_sb[:])

    # bulk weight loads ride the scalar engine's DGE queue
    whh0 = load("whh0", (32, 128), F32R, nc.scalar)
    whhd = load("whhd", (32, 128), eng=nc.scalar)
    wih1a = load("wih1a", (32, 128), F32R, nc.scalar)
    wih1b = load("wih1b", (32, 128), F32R, nc.scalar)
    b0 = load("b0", (128, 2), eng=nc.scalar)
    b1i_r = load("b1i_r", (1, 128), eng=nc.scalar)
    b1h_r = load("b1h_r", (1, 128), eng=nc.scalar)
    p1t = load("p1t", (52, 32), eng=nc.scalar)
    p2t = load("p2t", (52, 32), eng=nc.scalar)
    pb = load("pb", (32, 2), eng=nc.scalar)

    # whh1 augmented with layer-1 bias row (pairs with ones row 32 in h2)
    whh1 = sb.tile([33, 128], F32R, tag="whh1")
    nc.scalar.dma_start(whh1[0:32, :], A["whh1"][:])
    b1sum = sb.tile([1, 128], F32R, tag="b1sum")
    nc.vector.tensor_tensor(out=b1sum[:], in0=b1i_r[:], in1=b1h_r[:], op=OP.add)
    nc.scalar.dma_start(whh1[32:33, :], b1sum[:])

    wih0f = sb.tile([128, 8 * 128], F32R, tag="wih0f")
    wih0b = sb.tile([128, 8 * 128], F32R, tag="wih0b")
    nc.scalar.dma_start(wih0f[:].rearrange("p (k m) -> p k m", k=8),
                        A["wih0f"].rearrange("(k p) m -> p k m", p=128))
    nc.scalar.dma_start(wih0b[:].rearrange("p (k m) -> p k m", k=8),
                        A["wih0b"].rearrange("(k p) m -> p k m", p=128))

    # derived bias columns
    b0s = sb.tile([128, 1], F32, tag="b0s")
    nc.vector.tensor_tensor(out=b0s[:], in0=b0[:, 0:1], in1=b0[:, 1:2], op=OP.add)
    bds = sb.tile([32, 4], F32, tag="bds")
    nc.vector.tensor_tensor(out=bds[:], in0=bdg[:, 0:4], in1=bdg[:, 4:8], op=OP.add)
    bdh4 = sb.tile([32, 4], F32, tag="bdh4")
    nc.vector.tensor_tensor(out=bdh4[:], in0=bds[:], in1=half4[:], op=OP.mult)
    udh4 = sb.tile([32, 4], F32, tag="udh4")
    nc.vector.tensor_tensor(out=udh4[:], in0=bds[:], in1=wd_g[:], op=OP.subtract)
    nc.vector.tensor_tensor(out=udh4[:], in0=udh4[:], in1=half4[:], op=OP.mult)

    def gate_tanh(gpsum, n, cols, bias4, tagp):
        """Per-gate-group tanh: returns [Ti, Tf, To, Tg] (32, n) tiles."""
        ts_ = []
        for gi, gt in enumerate("ifog"):
            tt = sb_t.tile([32, n], F32, tag=f"{tagp}{gt}")
            nc.scalar.activation(
                tt[:], gpsum[GB[gt]:GB[gt] + 32, cols], AF.Tanh,
                bias=(bias4[:, gi:gi + 1] if bias4 is not None else 0.0),
                scale=(1.0 if gt == "g" else 0.5))
            ts_.append(tt)
        return ts_

    def cell_tail(Ts, n, c2_ap, h_out, tagp):
        """TC = tanh(c2/2); h_out = 0.5*(To+1)*TC."""
        tcx = sb_t.tile([32, n], F32, tag=f"{tagp}tc")
        nc.scalar.activation(tcx[:], c2_ap, AF.Tanh, scale=0.5)
        h2x = sb_t.tile([32, n], F32, tag=f"{tagp}h2")
        nc.vector.scalar_tensor_tensor(out=h2x[:], in0=Ts[2][:], scalar=1.0,
                                       in1=tcx[:], op0=OP.add, op1=OP.mult)
        nc.vector.tensor_scalar(h_out, h2x[:], 0.5, None, OP.mult)

    # ---------------- dec-zero path: dec_hT chunks (33 rows) ----------------
    # init pattern: rows 0:32 zero, row 32 ones (built in F32, cast-copied)
    z1 = sb.tile([33, 513], F32, tag="z1")
    nc.gpsimd.memset(z1[:], 0.0)
    nc.gpsimd.memset(z1[32:33, :], 1.0)
    dh = []
    for k in range(4):
        t = sb.tile([33, CH], F32R, tag=f"dh{k}")
        nc.vector.tensor_copy(t[:], z1[:, 0:CH])   # row 32 = ones bias row
        dh.append(t)

    for k in (1, 2, 3, 0):
        lo = 1 if k == 0 else 0
        n = CH - lo
        gz = ps_dz.tile([128, CH], F32, tag="gz")
        nc.tensor.matmul(gz[:, lo:CH], wd_r[:],
                         s_f[0:1, CH * k - 1 + lo:CH * (k + 1) - 1],
                         start=True, stop=True)
        Ts = gate_tanh(gz, n, slice(lo, CH), bdh4, "dz")
        z2 = sb_t.tile([32, n], F32, tag="dzz2")
        nc.vector.scalar_tensor_tensor(out=z2[:], in0=Ts[0][:], scalar=1.0,
                                       in1=Ts[3][:], op0=OP.add, op1=OP.mult)
        cell_tail(Ts, n, z2[:], dh[k][0:32, lo:CH], "dz")

    def gemm_tail(m):
        # chunks 1..3 (independent of the encoder) + DMA of cols 512:2048
        M = 128 if m < 31 else 32
        lsb = sb_l.tile([128, 3 * CH], F32, tag="lsb")
        for k in range(1, 4):
            pmm = ps_mm.tile([128, CH], F32, tag="pmm")
            nc.tensor.matmul(pmm[0:M, :], _r(wb)[:, 128 * m:128 * m + M],
                             _r(dh[k]), start=True, stop=True)
            if m % 2 == 0:
                nc.scalar.copy(lsb[0:M, CH * (k - 1):CH * k], pmm[0:M, :])
            else:
                nc.vector.tensor_copy(lsb[0:M, CH * (k - 1):CH * k], pmm[0:M, :])
        nc.sync.dma_start(A["logits_t"][128 * m:128 * m + M, CH:L], lsb[0:M, :])

    for m in range(6):          # keep PE busy while the U gathers land
        gemm_tail(m)

    # ---------------- U windows: gather + transpose + masked GEMMs ----------
    # window cols 0:256 = t 0:256 (head), 256:512 = t 1792:2048 (tail)
    xt = sb.tile([128, 8 * 512], F32R, tag="xt")
    for j in range(4):
        xg = sb_g.tile([128, 1024], F32, tag="xg")
        nc.gpsimd.indirect_dma_start(
            out=xg[:], out_offset=None, in_=A["emb"][:],
            in_offset=bass.IndirectOffsetOnAxis(ap=xw[:, j:j + 1], axis=0))
        for k in range(8):
            ptr = ps_tr.tile([128, 128], F32, tag="ptr")
            nc.tensor.transpose(ptr[:], xg[:, 128 * k:128 * (k + 1)], identf[:])
            nc.vector.tensor_copy(xt[:, 512 * k + 128 * j:512 * k + 128 * j + 128],
                                  ptr[:])
    xtr = sb.tile([128, 8 * 512], F32R, tag="xtr")     # window-reversed copy
    for k in range(8):
        nc.vector.tensor_copy(xtr[:, 512 * k:512 * (k + 1)],
                              xt[:, 512 * (k + 1) - 1::-1][:, 0:512])

    # merged U psum (128, 384): cols 0:256 = I0 (A fwd | B bwd),
    # cols 256:384 = I1 (C fwd | D bwd).  fwd part from xt, bwd from xtr:
    #   A: tail fwd = xt[0:256]      C: head fwd = xt[256:384]
    #   B: head rev = xtr[0:256]     D: tail rev = xtr[256:384]
    up = ps_u.tile([128, 384], F32, tag="up")
    for k in range(8):
        nc.tensor.matmul(up[:], _r(wih0f)[:, 128 * k:128 * (k + 1)],
                         _r(xt)[:, 512 * k:512 * k + 384],
                         start=(k == 0), stop=False)
    for k in range(8):
        nc.tensor.matmul(up[:], _r(wih0b)[:, 128 * k:128 * (k + 1)],
                         _r(xtr)[:, 512 * k:512 * k + 384],
                         start=False, stop=(k == 7))
    u0 = sb.tile([128, 256], F32R, tag="u0")
    nc.vector.tensor_scalar(u0[:], up[:, 0:256], b0s[:, :1], None, OP.add)
    u1 = sb.tile([128, 128], F32R, tag="u1")
    nc.vector.tensor_scalar(u1[:], up[:, 256:384], b0s[:, :1], None, OP.add)

    def gemm_head(m):
        # chunk 0 (needs out0 in dh[0] col 0) + DMA of cols 0:512
        M = 128 if m < 31 else 32
        lsb0 = sb_l.tile([128, CH], F32, tag="lsb0")
        pmm = ps_mm.tile([128, CH], F32, tag="pmm")
        nc.tensor.matmul(pmm[0:M, :], _r(wb)[:, 128 * m:128 * m + M],
                         _r(dh[0]), start=True, stop=True)
        if m % 2 == 0:
            nc.scalar.copy(lsb0[0:M, :], pmm[0:M, :])
        else:
            nc.vector.tensor_copy(lsb0[0:M, :], pmm[0:M, :])
        nc.sync.dma_start(A["logits_t"][128 * m:128 * m + M, 0:CH], lsb0[0:M, :])

    # ---------------- embs output gather ----------------
    for j in range(2):
        et = sb_e.tile([128, 1024], F32, tag="et")
        nc.gpsimd.indirect_dma_start(
            out=et[:], out_offset=None, in_=A["emb"][:],
            in_offset=bass.IndirectOffsetOnAxis(ap=xmy[:, j:j + 1], axis=0))
        nc.sync.dma_start(A["embs_my"][128 * j:128 * (j + 1), :], et[:])

    # ---------------- joint fixed point (GEMM tiles interleaved) -----------
    h0 = sb.tile([32, 257], F32R, tag="h0")
    h1 = sb.tile([32, 129], F32R, tag="h1")
    h1r = sb.tile([32, 129], F32R, tag="h1r")
    h2 = sb.tile([33, 129], F32R, tag="h2")    # row 32 = ones (whh1 bias row)
    nc.vector.tensor_copy(h0[:], z1[0:32, 0:257])
    nc.vector.tensor_copy(h1[:], z1[0:32, 0:129])
    nc.vector.tensor_copy(h1r[:], z1[0:32, 0:129])
    nc.vector.tensor_copy(h2[:], z1[0:33, 0:129])  # row 32 = ones (whh1 bias)
    c2_0 = sb.tile([32, 256], F32, tag="c2_0")
    c2_12 = sb.tile([32, 256], F32, tag="c2_12")

    for it in range(NITER):
        # --- instance 0 (A+B paired, 256 cols) ---
        g0 = ps_u.tile([128, 384], F32, tag="up")
        nc.tensor.matmul(g0[:, 0:256], _r(ident), _r(u0), start=True, stop=False)
        nc.tensor.matmul(g0[:, 0:256], _r(whh0), _r(h0)[:, 0:256], start=False, stop=True)
        Ts = gate_tanh(g0, 256, slice(0, 256), None, "i0")
        sf0 = sb_t.tile([32, 256], F32, tag="i0sf")
        nc.vector.tensor_scalar(sf0[:], Ts[1][:], 0.5, 0.5, OP.mult, OP.add)
        z20 = sb_t.tile([32, 256], F32, tag="i0z2")
        nc.vector.scalar_tensor_tensor(out=z20[:], in0=Ts[0][:], scalar=1.0,
                                       in1=Ts[3][:], op0=OP.add, op1=OP.mult)
        nc.vector.tensor_tensor_scan(c2_0[:], sf0[:], z20[:], 0.0,
                                     OP.mult, OP.add)
        cell_tail(Ts, 256, c2_0[:], h0[:, 1:257], "i0")

        # --- instances 1+2 share G (cols 0:128 = I1=C+D, 128:256 = I2=E+F) ---
        nc.vector.tensor_copy(h1r[:], h1[:, 128::-1])
        g12 = ps_g.tile([128, 256], F32, tag="g12")
        nc.tensor.matmul(g12[:, 0:128], _r(ident), _r(u1), start=True, stop=False)
        nc.tensor.matmul(g12[:, 0:128], _r(whh0), _r(h1)[:, 0:128],
                         start=False, stop=True)
        nc.tensor.matmul(g12[:, 128:256], _r(wih1a), _r(h0)[:, 129:257],
                         start=True, stop=False)
        nc.tensor.matmul(g12[:, 128:256], _r(wih1b), _r(h1r)[:, 0:128],
                         start=False, stop=False)
        nc.tensor.matmul(g12[:, 128:256], _r(whh1), _r(h2)[:, 0:128],
                         start=False, stop=True)
        Ts = gate_tanh(g12, 256, slice(0, 256), None, "i12")
        sf12 = sb_t.tile([32, 256], F32, tag="i12sf")
        nc.vector.tensor_scalar(sf12[:], Ts[1][:], 0.5, 0.5, OP.mult, OP.add)
        z212 = sb_t.tile([32, 256], F32, tag="i12z2")
        nc.vector.scalar_tensor_tensor(out=z212[:], in0=Ts[0][:], scalar=1.0,
                                       in1=Ts[3][:], op0=OP.add, op1=OP.mult)
        nc.vector.tensor_tensor_scan(c2_12[:, 0:128], sf12[:, 0:128],
                                     z212[:, 0:128], 0.0, OP.mult, OP.add)
        nc.vector.tensor_tensor_scan(c2_12[:, 128:256], sf12[:, 128:256],
                                     z212[:, 128:256], 0.0, OP.mult, OP.add)
        tc12 = sb_t.tile([32, 256], F32, tag="i12tc")
        nc.scalar.activation(tc12[:], c2_12[:], AF.Tanh, scale=0.5)
        h2x12 = sb_t.tile([32, 256], F32, tag="i12h2x")
        nc.vector.scalar_tensor_tensor(out=h2x12[:], in0=Ts[2][:], scalar=1.0,
                                       in1=tc12[:], op0=OP.add, op1=OP.mult)
        nc.vector.tensor_scalar(h1[:, 1:129], h2x12[:, 0:128], 0.5, None, OP.mult)
        nc.vector.tensor_scalar(h2[0:32, 1:129], h2x12[:, 128:256], 0.5,
                                None, OP.mult)
        for m in range(6 + 2 * it, 6 + 2 * it + 2):
            gemm_tail(m)

    for m in range(6 + 2 * NITER, 6 + 2 * NITER + 6):
        gemm_tail(m)

    # ---------------- finals -> projections -> decoder step 0 ----------------
    hc = sb.tile([52, 2], F32, tag="hc")     # [hfA;hbB;hfE;hbF] | c_flat
    nc.sync.dma_start(hc[0:13, 0:1], h0[0:13, 256:257].bitcast(F32))
    nc.scalar.dma_start(hc[13:26, 0:1], h0[16:29, 256:257].bitcast(F32))
    nc.sync.dma_start(hc[26:39, 0:1], h2[0:13, 128:129].bitcast(F32))
    nc.scalar.dma_start(hc[39:52, 0:1], h2[16:29, 128:129].bitcast(F32))
    nc.sync.dma_start(hc[0:13, 1:2], c2_0[0:13, 255:256])
    nc.scalar.dma_start(hc[13:26, 1:2], c2_0[16:29, 255:256])
    nc.sync.dma_start(hc[26:39, 1:2], c2_12[0:13, 255:256])
    nc.scalar.dma_start(hc[39:52, 1:2], c2_12[16:29, 255:256])
    nc.vector.tensor_scalar(hc[:, 1:2], hc[:, 1:2], 0.5, None, OP.mult)

    scr = ps_ms.tile([128, 512], F32, tag="gz")
    nc.tensor.matmul(scr[0:32, 110:111], p1t[:], hc[:, 0:1],
                     start=True, stop=True)
    nc.tensor.matmul(scr[0:32, 111:112], p2t[:], hc[:, 1:2],
                     start=True, stop=True)
    st = sb.tile([32, 2], F32, tag="st")
    nc.vector.tensor_tensor(out=st[:], in0=scr[0:32, 110:112], in1=pb[:], op=OP.add)
    nc.tensor.matmul(scr[0:128, 120:121], whhd[:], st[:, 0:1],
                     start=True, stop=True)
    Tsd = gate_tanh(scr, 1, slice(120, 121), udh4, "d0")
    z2d = sb.tile([32, 1], F32, tag="z2d")
    nc.vector.scalar_tensor_tensor(out=z2d[:], in0=Tsd[0][:], scalar=1.0,
                                   in1=Tsd[3][:], op0=OP.add, op1=OP.mult)
    fc2d = sb.tile([32, 1], F32, tag="fc2d")
    nc.vector.scalar_tensor_tensor(out=fc2d[:], in0=Tsd[1][:], scalar=1.0,
                                   in1=st[:, 1:2], op0=OP.add, op1=OP.mult)
    c2d = sb.tile([32, 1], F32, tag="c2d")
    nc.vector.tensor_tensor(out=c2d[:], in0=z2d[:], in1=fc2d[:], op=OP.add)
    o0 = sb.tile([33, 1], F32, tag="o0")
    nc.gpsimd.memset(o0[:], 1.0)                 # row 32 stays 1 (bias row)
    cell_tail(Tsd, 1, c2d[:], o0[0:32, :], "d0")

    # out0 becomes dec_h column 0 -> logits row 0 comes out of the main GEMM
    nc.vector.tensor_copy(dh[0][0:32, 0:1], o0[0:32, :])

    # ---------------- main GEMM: leftover m-tiles ----------------
    # leftover tails and heads interleaved to keep PE dense after the encoder
    heads = list(range(32))
    tails = list(range(6 + 2 * NITER + 6, 32))
    while heads or tails:
        if tails:
            gemm_tail(tails.pop(0))
        if heads:
            gemm_head(heads.pop(0))


def _host_prep(inputs):
    x = np.ascontiguousarray(np.asarray(inputs["x"], dtype=np.int32))
    emb = np.ascontiguousarray(np.asarray(inputs["emb"], dtype=np.float32))
    l0, l1 = inputs["enc_params"]
    wih0, whh0, bi0, bh0 = _pack_dir(l0[0], l0[1])
    wih1, whh1, bi1, bh1 = _pack_dir(l1[0], l1[1])
    wihd, whhd, bid_, bhd = _pack_dir(inputs["dec_params"][0], inputs["dec_params"][1])
    out_w = np.asarray(inputs["out_w"], dtype=np.float32)
    out_b = np.asarray(inputs["out_b"], dtype=np.float32)

    fm, bm = _dir_col_mask(0), _dir_col_mask(1)
    wih0f = np.ascontiguousarray((wih0 * fm[:, None]).T)
    wih0b = np.ascontiguousarray((wih0 * bm[:, None]).T)
    wih1_l = np.ascontiguousarray(wih1.T)                # (26, 128)
    wih1_p = np.zeros((32, 128), np.float32)
    wih1_p[0:13] = wih1_l[0:13]                          # yf rows
    wih1_p[16:29] = wih1_l[13:26]                        # yb rows
    yf_rows = np.zeros(32, np.float32); yf_rows[0:13] = 1
    yb_rows = np.zeros(32, np.float32); yb_rows[16:29] = 1
    wih1a = wih1_p * (yf_rows[:, None] * fm[None, :] + yb_rows[:, None] * bm[None, :])
    wih1b = wih1_p * (yb_rows[:, None] * fm[None, :] + yf_rows[:, None] * bm[None, :])

    wb_full = np.zeros((33, 32000), np.float32)
    wb_full[0:13] = out_w.T[0:13]
    wb_full[16:29] = out_w.T[13:26]
    wb_full[32] = out_b

    def proj_pack(w):
        w = np.asarray(w, np.float32)                    # (26, 52)
        p = np.zeros((52, 32), np.float32)
        p[:, 0:13] = w[0:13].T
        p[:, 16:29] = w[13:26].T
        return np.ascontiguousarray(p)

    def bias_pack32(b):
        b = np.asarray(b, np.float32)
        p = np.zeros(32, np.float32)
        p[0:13] = b[0:13]; p[16:29] = b[13:26]
        return p

    shared = {
        "x": x.reshape(1, L),
        "x_win": np.concatenate([x[L - 2 * W:L], x[0:2 * W]]),
        "emb": emb,
        "wih0f": wih0f, "wih0b": wih0b,
        "wih1a": np.ascontiguousarray(wih1a), "wih1b": np.ascontiguousarray(wih1b),
        "whh0": np.ascontiguousarray(whh0), "whh1": np.ascontiguousarray(whh1),
        "whhd": np.ascontiguousarray(whhd),
        "b0": np.stack([bi0, bh0], 1),
        "b1i_r": np.ascontiguousarray(bi1[None, :]),
        "b1h_r": np.ascontiguousarray(bh1[None, :]),
        "bdg": np.ascontiguousarray(
            np.concatenate([_group_cols(bid_), _group_cols(bhd)], 1)),
        "wd_g": np.ascontiguousarray(_group_cols(wihd[:, 0])),
        "wd_r": np.ascontiguousarray(wihd.T),            # (1, 128)
        "p1t": proj_pack(inputs["proj1_w"]), "p2t": proj_pack(inputs["proj2_w"]),
        "pb": np.stack([bias_pack32(inputs["proj1_b"]),
                        bias_pack32(inputs["proj2_b"])], 1),
    }
    per_core = []
    for c in range(NCORE):
        m = dict(shared)
        m["x_my"] = np.ascontiguousarray(x[ESH * c:ESH * (c + 1)])
        m["wb"] = np.ascontiguousarray(wb_full[:, VSH * c:VSH * (c + 1)])
        per_core.append(m)
    return per_core


def _declare(nc):
    A = {}
    spec = {
        "x": ((1, L), I32), "x_win": ((4 * W,), I32), "x_my": ((ESH,), I32),
        "emb": ((32001, 1024), F32),
        "wih0f": ((1024, 128), F32R), "wih0b": ((1024, 128), F32R),
        "wih1a": ((32, 128), F32R), "wih1b": ((32, 128), F32R),
        "whh0": ((32, 128), F32R), "whh1": ((32, 128), F32R), "whhd": ((32, 128), F32),
        "b0": ((128, 2), F32), "b1i_r": ((1, 128), F32), "b1h_r": ((1, 128), F32),
        "bdg": ((32, 8), F32), "wd_g": ((32, 4), F32), "wd_r": ((1, 128), F32),
        "p1t": ((52, 32), F32), "p2t": ((52, 32), F32), "pb": ((32, 2), F32),
        "wb": ((33, VSH), F32R),
    }
    for name, (shape, dt) in spec.items():
        A[name] = nc.dram_tensor(name, list(shape), dt, kind="ExternalInput").ap()
    for name, shape in (("logits_t", (VSH, L)),
                        ("embs_my", (ESH, 1024))):
        A[name] = nc.dram_tensor(name, list(shape), F32, kind="ExternalOutput").ap()
    return A


def build_nc():
    nc = bacc.Bacc("TRN2", target_bir_lowering=False, debug=False,
                   num_devices=NCORE)
    A = _declare(nc)
    with tile.TileContext(nc) as tc:
        with ExitStack() as ctx:
            _emit(ctx, tc, A)
    nc.compile()
    return nc


def kernel(_trace=False, _bench=None, **inputs):
    in_maps = _host_prep(inputs)
    nc = build_nc()
    nc.m = get_hw_module(nc.m)
    res = bass_utils.run_bass_kernel_spmd(
        nc, in_maps, core_ids=list(range(NCORE)), trace=_trace)
    if _bench is not None:
        _bench["exec_time_ns"] = res.exec_time_ns
        _bench["res"] = res
    logits = np.empty((L, 32000), np.float32)
    embs = np.empty((L, 1024), np.float32)
    for c in range(NCORE):
        out = res.results[c]
        logits[:, VSH * c:VSH * (c + 1)] = out["logits_t"].T
        embs[ESH * c:ESH * (c + 1)] = out["embs_my"]
    return logits, embs


# revision 37
# speedup vs baseline: 1.1642x; 1.1642x over previous
"""Trainium2 Bass kernel for nn_AE_rnn (bi-LSTM autoencoder over vocab logits).

Strategy (8 NeuronCores, SPMD, no collectives):
- logits GEMM (2048x26 @ 26x32000, 262MB out) sharded over vocab: each core
  computes a (4000, 2048) transposed logits shard via f32r matmuls with the
  bias folded in as an extra contraction row, PSUM -> SBUF -> DRAM.
- rows 1..2047 of dec_h come from the zero-state decoder path (elementwise,
  cheap, available immediately) so the big GEMM starts at t=0 of the kernel.
- row 0 needs the encoder. The 4 bidirectional LSTM scans are replaced by a
  truncated-window (W=128, forget-gate decay) joint fixed-point iteration:
  gates from the previous h estimate, cell state via the DVE
  tensor_tensor_scan linear-recurrence instruction, 8 iterations to fp32
  accuracy. Chains are packed as columns; sigmoid via tanh half-angle so the
  scalar engine only ever computes tanh. Encoder replicated on every core
  (no collectives); each core computes logits row 0 for its own vocab shard.
- embeddings output gathered row-sharded (256 rows/core) via indirect DMA.

Hardware layout constraints honored here:
- engine SBUF access patterns must start at partition 0/32/64/96, and all
  SBUF operands of one vector op must share the same start partition. So the
  13-row gate groups are padded into 32-row blocks (gate tiles: 128 rows
  [i|f|o|g], fwd at +0:13, bwd at +16:29 of each block, pads stay zero), the
  tanh of each gate block gets its own 32-row tile (PSUM input slices are
  exempt from the same-start rule), and the cell update works on the doubled
  state c2=2c:  z2 = (Ti+1)*Tg,  c2 = scan(sf, z2),  h = 0.5*(To+1)*tanh(c2/2)
  so every vector op sees operands based at partition 0.

kernel(**inputs) takes the full unsharded inputs and returns (logits, embs).
"""
from contextlib import ExitStack

import numpy as np

import concourse.bass as bass
import concourse.tile as tile
from concourse import bacc, mybir
from concourse import bass_utils
from concourse.bass_interp import get_hw_module
from concourse.masks import make_identity

F32 = mybir.dt.float32
F32R = mybir.dt.float32r
I32 = mybir.dt.int32
AF = mybir.ActivationFunctionType
OP = mybir.AluOpType

L = 2048
H = 13
W = 128            # live window per chain
NITER = 5
NCORE = 8
VSH = 32000 // NCORE       # 4000 vocab rows per core
ESH = L // NCORE           # 256 embs rows per core
CH = 512                   # t-chunk for dec_h / GEMM

GB = {"i": 0, "f": 32, "o": 64, "g": 96}   # gate block starts (128 rows)
TORCH = {"i": 0, "f": 13, "g": 26, "o": 39}


def _r(t):
    return t[:].bitcast(F32R)


def _pack_dir(p_f, p_b):
    """Pack one (fwd,bwd) LSTM cell pair into the padded block layout.
    Returns wih (128, in_sz), whh (32, 128), bih (128,), bhh (128,)."""
    in_sz = np.asarray(p_f[0]).shape[1]
    wih = np.zeros((128, in_sz), np.float32)
    whh = np.zeros((32, 128), np.float32)
    bih_p = np.zeros(128, np.float32)
    bhh_p = np.zeros(128, np.float32)
    for d, p in ((0, p_f), (1, p_b)):
        Wih, Whh, bih, bhh = [np.asarray(a, dtype=np.float32) for a in p]
        for gt in "ifog":
            src = slice(TORCH[gt], TORCH[gt] + H)
            dst = slice(GB[gt] + 16 * d, GB[gt] + 16 * d + H)
            wih[dst] = Wih[src]
            whh[16 * d:16 * d + H, dst] = Whh[src].T
            bih_p[dst] = bih[src]
            bhh_p[dst] = bhh[src]
    return wih, whh, bih_p, bhh_p


def _dir_col_mask(d):
    m = np.zeros(128, np.float32)
    for gt in "ifog":
        m[GB[gt] + 16 * d:GB[gt] + 16 * d + H] = 1.0
    return m


def _group_cols(vec128):
    """(128,) packed vector -> (32, 4) one column per gate group [i,f,o,g]."""
    return np.stack([vec128[GB[gt]:GB[gt] + 32] for gt in "ifog"], 1)


def _emit(ctx: ExitStack, tc: tile.TileContext, A: dict):
    nc = tc.nc
    sb = ctx.enter_context(tc.tile_pool(name="sb", bufs=1))
    sb_g = ctx.enter_context(tc.tile_pool(name="sb_g", bufs=2))
    sb_l = ctx.enter_context(tc.tile_pool(name="sb_l", bufs=3))
    sb_e = ctx.enter_context(tc.tile_pool(name="sb_e", bufs=2))
    sb_t = ctx.enter_context(tc.tile_pool(name="sb_t", bufs=2))
    ps_tr = ctx.enter_context(tc.tile_pool(name="ps_tr", bufs=1, space="PSUM"))
    ps_u = ctx.enter_context(tc.tile_pool(name="ps_u", bufs=1, space="PSUM"))
    ps_g = ctx.enter_context(tc.tile_pool(name="ps_g", bufs=1, space="PSUM"))
    ps_dz = ctx.enter_context(tc.tile_pool(name="ps_dz", bufs=1, space="PSUM"))
    ps_ms = ps_dz
    ps_mm = ctx.enter_context(tc.tile_pool(name="ps_mm", bufs=3, space="PSUM"))

    # ---------------- constants + weight loads ----------------
    identf = sb.tile([128, 128], F32, tag="identf")
    make_identity(nc, identf[:])
    ident = sb.tile([128, 128], F32R, tag="ident")
    nc.vector.tensor_copy(ident[:], identf[:])
    half4 = sb.tile([32, 4], F32, tag="half4")
    nc.gpsimd.memset(half4[:], 1.0)
    nc.gpsimd.memset(half4[:, 0:3], 0.5)     # i,f,o halved; g not

    def load(name, shape, dtype=F32, eng=None):
        t = sb.tile(list(shape), dtype, tag=name)
        (eng or nc.sync).dma_start(t[:], A[name][:])
        return t

    # dec-zero / early-GEMM critical loads first on the sync queue
    x_sb = sb.tile([1, L], I32, tag="x_sb")
    nc.sync.dma_start(x_sb[:], A["x"][:])
    wd_r = load("wd_r", (1, 128))
    bdg = load("bdg", (32, 8))
    wd_g = load("wd_g", (32, 4))
    wb = load("wb", (33, VSH), F32R)
    xw = sb.tile([128, 4], I32, tag="xw")
    nc.sync.dma_start(xw[:], A["x_win"].rearrange("(c p) -> p c", p=128))
    xmy = sb.tile([128, 2], I32, tag="xmy")
    nc.sync.dma_start(xmy[:], A["x_my"].rearrange("(c p) -> p c", p=128))
    s_f = sb.tile([1, L], F32, tag="s_f")
    nc.vector.tensor_copy(s_f[:], x_sb[:])

    # bulk weight loads ride the scalar engine's DGE queue
    whh0 = load("whh0", (32, 128), F32R, nc.scalar)
    whhd = load("whhd", (32, 128), eng=nc.scalar)
    wih1a = load("wih1a", (32, 128), F32R, nc.scalar)
    wih1b = load("wih1b", (32, 128), F32R, nc.scalar)
    b0 = load("b0", (128, 2), eng=nc.scalar)
    b1i_r = load("b1i_r", (1, 128), eng=nc.scalar)
    b1h_r = load("b1h_r", (1, 128), eng=nc.scalar)
    p1t = load("p1t", (52, 32), eng=nc.scalar)
    p2t = load("p2t", (52, 32), eng=nc.scalar)
    pb = load("pb", (32, 2), eng=nc.scalar)

    # whh1 augmented with layer-1 bias row (pairs with ones row 32 in h2)
    whh1 = sb.tile([33, 128], F32R, tag="whh1")
    nc.scalar.dma_start(whh1[0:32, :], A["whh1"][:])
    b1sum = sb.tile([1, 128], F32R, tag="b1sum")
    nc.vector.tensor_tensor(out=b1sum[:], in0=b1i_r[:], in1=b1h_r[:], op=OP.add)
    nc.scalar.dma_start(whh1[32:33, :], b1sum[:])

    wih0f = sb.tile([128, 8 * 128], F32R, tag="wih0f")
    wih0b = sb.tile([128, 8 * 128], F32R, tag="wih0b")
    nc.scalar.dma_start(wih0f[:].rearrange("p (k m) -> p k m", k=8),
                        A["wih0f"].rearrange("(k p) m -> p k m", p=128))
    nc.scalar.dma_start(wih0b[:].rearrange("p (k m) -> p k m", k=8),
                        A["wih0b"].rearrange("(k p) m -> p k m", p=128))

    # derived bias columns
    b0s = sb.tile([128, 1], F32, tag="b0s")
    nc.vector.tensor_tensor(out=b0s[:], in0=b0[:, 0:1], in1=b0[:, 1:2], op=OP.add)
    bds = sb.tile([32, 4], F32, tag="bds")
    nc.vector.tensor_tensor(out=bds[:], in0=bdg[:, 0:4], in1=bdg[:, 4:8], op=OP.add)
    bdh4 = sb.tile([32, 4], F32, tag="bdh4")
    nc.vector.tensor_tensor(out=bdh4[:], in0=bds[:], in1=half4[:], op=OP.mult)
    udh4 = sb.tile([32, 4], F32, tag="udh4")
    nc.vector.tensor_tensor(out=udh4[:], in0=bds[:], in1=wd_g[:], op=OP.subtract)
    nc.vector.tensor_tensor(out=udh4[:], in0=udh4[:], in1=half4[:], op=OP.mult)

    def gate_tanh(gpsum, n, cols, bias4, tagp):
        """Per-gate-group tanh: returns [Ti, Tf, To, Tg] (32, n) tiles."""
        ts_ = []
        for gi, gt in enumerate("ifog"):
            tt = sb_t.tile([32, n], F32, tag=f"{tagp}{gt}")
            nc.scalar.activation(
                tt[:], gpsum[GB[gt]:GB[gt] + 32, cols], AF.Tanh,
                bias=(bias4[:, gi:gi + 1] if bias4 is not None else 0.0),
                scale=(1.0 if gt == "g" else 0.5))
            ts_.append(tt)
        return ts_

    def cell_tail(Ts, n, c2_ap, h_out, tagp):
        """TC = tanh(c2/2); h_out = 0.5*(To+1)*TC."""
        tcx = sb_t.tile([32, n], F32, tag=f"{tagp}tc")
        nc.scalar.activation(tcx[:], c2_ap, AF.Tanh, scale=0.5)
        h2x = sb_t.tile([32, n], F32, tag=f"{tagp}h2")
        nc.vector.scalar_tensor_tensor(out=h2x[:], in0=Ts[2][:], scalar=1.0,
                                       in1=tcx[:], op0=OP.add, op1=OP.mult)
        nc.vector.tensor_scalar(h_out, h2x[:], 0.5, None, OP.mult)

    # ---------------- dec-zero path: dec_hT chunks (33 rows) ----------------
    # init pattern: rows 0:32 zero, row 32 ones (built in F32, cast-copied)
    z1 = sb.tile([33, 513], F32, tag="z1")
    nc.gpsimd.memset(z1[:], 0.0)
    nc.gpsimd.memset(z1[32:33, :], 1.0)
    dh = []
    for k in range(4):
        t = sb.tile([33, CH], F32R, tag=f"dh{k}")
        nc.vector.tensor_copy(t[:], z1[:, 0:CH])   # row 32 = ones bias row
        dh.append(t)

    for k in (1, 2, 3, 0):
        lo = 1 if k == 0 else 0
        n = CH - lo
        gz = ps_dz.tile([128, CH], F32, tag="gz")
        nc.tensor.matmul(gz[:, lo:CH], wd_r[:],
                         s_f[0:1, CH * k - 1 + lo:CH * (k + 1) - 1],
                         start=True, stop=True)
        Ts = gate_tanh(gz, n, slice(lo, CH), bdh4, "dz")
        z2 = sb_t.tile([32, n], F32, tag="dzz2")
        nc.vector.scalar_tensor_tensor(out=z2[:], in0=Ts[0][:], scalar=1.0,
                                       in1=Ts[3][:], op0=OP.add, op1=OP.mult)
        cell_tail(Ts, n, z2[:], dh[k][0:32, lo:CH], "dz")

    def gemm_tail(m):
        # chunks 1..3 (independent of the encoder) + DMA of cols 512:2048
        M = 128 if m < 31 else 32
        lsb = sb_l.tile([128, 3 * CH], F32, tag="lsb")
        for k in range(1, 4):
            pmm = ps_mm.tile([128, CH], F32, tag="pmm")
            nc.tensor.matmul(pmm[0:M, :], _r(wb)[:, 128 * m:128 * m + M],
                             _r(dh[k]), start=True, stop=True)
            if m % 2 == 0:
                nc.scalar.copy(lsb[0:M, CH * (k - 1):CH * k], pmm[0:M, :])
            else:
                nc.vector.tensor_copy(lsb[0:M, CH * (k - 1):CH * k], pmm[0:M, :])
        nc.sync.dma_start(A["logits_t"][128 * m:128 * m + M, CH:L], lsb[0:M, :])

    for m in range(6):          # keep PE busy while the U gathers land
        gemm_tail(m)

    # ---------------- U windows: gather + transpose + masked GEMMs ----------
    # window cols 0:256 = t 0:256 (head), 256:512 = t 1792:2048 (tail)
    xt = sb.tile([128, 8 * 512], F32R, tag="xt")
    for j in range(4):
        xg = sb_g.tile([128, 1024], F32, tag="xg")
        nc.gpsimd.indirect_dma_start(
            out=xg[:], out_offset=None, in_=A["emb"][:],
            in_offset=bass.IndirectOffsetOnAxis(ap=xw[:, j:j + 1], axis=0))
        for k in range(8):
            ptr = ps_tr.tile([128, 128], F32, tag="ptr")
            nc.tensor.transpose(ptr[:], xg[:, 128 * k:128 * (k + 1)], identf[:])
            nc.vector.tensor_copy(xt[:, 512 * k + 128 * j:512 * k + 128 * j + 128],
                                  ptr[:])
    xtr = sb.tile([128, 8 * 512], F32R, tag="xtr")     # window-reversed copy
    for k in range(8):
        nc.vector.tensor_copy(xtr[:, 512 * k:512 * (k + 1)],
                              xt[:, 512 * (k + 1) - 1::-1][:, 0:512])

    # merged U psum (128, 384): cols 0:256 = I0 (A fwd | B bwd),
    # cols 256:384 = I1 (C fwd | D bwd).  fwd part from xt, bwd from xtr:
    #   A: tail fwd = xt[0:256]      C: head fwd = xt[256:384]
    #   B: head rev = xtr[0:256]     D: tail rev = xtr[256:384]
    up = ps_u.tile([128, 384], F32, tag="up")
    for k in range(8):
        nc.tensor.matmul(up[:], _r(wih0f)[:, 128 * k:128 * (k + 1)],
                         _r(xt)[:, 512 * k:512 * k + 384],
                         start=(k == 0), stop=False)
    for k in range(8):
        nc.tensor.matmul(up[:], _r(wih0b)[:, 128 * k:128 * (k + 1)],
                         _r(xtr)[:, 512 * k:512 * k + 384],
                         start=False, stop=(k == 7))
    u0 = sb.tile([128, 256], F32R, tag="u0")
    nc.vector.tensor_scalar(u0[:], up[:, 0:256], b0s[:, :1], None, OP.add)
    u1 = sb.tile([128, 128], F32R, tag="u1")
    nc.vector.tensor_scalar(u1[:], up[:, 256:384], b0s[:, :1], None, OP.add)

    def gemm_head(m):
        # chunk 0 (needs out0 in dh[0] col 0) + DMA of cols 0:512
        M = 128 if m < 31 else 32
        lsb0 = sb_l.tile([128, CH], F32, tag="lsb0")
        pmm = ps_mm.tile([128, CH], F32, tag="pmm")
        nc.tensor.matmul(pmm[0:M, :], _r(wb)[:, 128 * m:128 * m + M],
                         _r(dh[0]), start=True, stop=True)
        if m % 2 == 0:
            nc.scalar.copy(lsb0[0:M, :], pmm[0:M, :])
        else:
            nc.vector.tensor_copy(lsb0[0:M, :], pmm[0:M, :])
        nc.sync.dma_start(A["logits_t"][128 * m:128 * m + M, 0:CH], lsb0[0:M, :])

    # ---------------- embs output gather ----------------
    for j in range(2):
        et = sb_e.tile([128, 1024], F32, tag="et")
        nc.gpsimd.indirect_dma_start(
            out=et[:], out_offset=None, in_=A["emb"][:],
            in_offset=bass.IndirectOffsetOnAxis(ap=xmy[:, j:j + 1], axis=0))
        nc.sync.dma_start(A["embs_my"][128 * j:128 * (j + 1), :], et[:])

    # ---------------- joint fixed point (GEMM tiles interleaved) -----------
    h0 = sb.tile([32, 257], F32R, tag="h0")
    h1 = sb.tile([32, 129], F32R, tag="h1")
    h1r = sb.tile([32, 129], F32R, tag="h1r")
    h2 = sb.tile([33, 129], F32R, tag="h2")    # row 32 = ones (whh1 bias row)
    nc.vector.tensor_copy(h0[:], z1[0:32, 0:257])
    nc.vector.tensor_copy(h1[:], z1[0:32, 0:129])
    nc.vector.tensor_copy(h1r[:], z1[0:32, 0:129])
    nc.vector.tensor_copy(h2[:], z1[0:33, 0:129])  # row 32 = ones (whh1 bias)
    c2_0 = sb.tile([32, 256], F32, tag="c2_0")
    c2_12 = sb.tile([32, 256], F32, tag="c2_12")

    for it in range(NITER):
        # --- instance 0 (A+B paired, 256 cols) ---
        g0 = ps_g.tile([128, 256], F32, tag="g0")
        nc.tensor.matmul(g0[:], _r(ident), _r(u0), start=True, stop=False)
        nc.tensor.matmul(g0[:], _r(whh0), _r(h0)[:, 0:256], start=False, stop=True)
        Ts = gate_tanh(g0, 256, slice(0, 256), None, "i0")
        sf0 = sb_t.tile([32, 256], F32, tag="i0sf")
        nc.vector.tensor_scalar(sf0[:], Ts[1][:], 0.5, 0.5, OP.mult, OP.add)
        z20 = sb_t.tile([32, 256], F32, tag="i0z2")
        nc.vector.scalar_tensor_tensor(out=z20[:], in0=Ts[0][:], scalar=1.0,
                                       in1=Ts[3][:], op0=OP.add, op1=OP.mult)
        nc.vector.tensor_tensor_scan(c2_0[:], sf0[:], z20[:], 0.0,
                                     OP.mult, OP.add)
        cell_tail(Ts, 256, c2_0[:], h0[:, 1:257], "i0")

        # --- instances 1+2 share G (cols 0:128 = I1=C+D, 128:256 = I2=E+F) ---
        nc.vector.tensor_copy(h1r[:], h1[:, 128::-1])
        g12 = ps_g.tile([128, 256], F32, tag="g12")
        nc.tensor.matmul(g12[:, 0:128], _r(ident), _r(u1), start=True, stop=False)
        nc.tensor.matmul(g12[:, 0:128], _r(whh0), _r(h1)[:, 0:128],
                         start=False, stop=True)
        nc.tensor.matmul(g12[:, 128:256], _r(wih1a), _r(h0)[:, 129:257],
                         start=True, stop=False)
        nc.tensor.matmul(g12[:, 128:256], _r(wih1b), _r(h1r)[:, 0:128],
                         start=False, stop=False)
        nc.tensor.matmul(g12[:, 128:256], _r(whh1), _r(h2)[:, 0:128],
                         start=False, stop=True)
        Ts = gate_tanh(g12, 256, slice(0, 256), None, "i12")
        sf12 = sb_t.tile([32, 256], F32, tag="i12sf")
        nc.vector.tensor_scalar(sf12[:], Ts[1][:], 0.5, 0.5, OP.mult, OP.add)
        z212 = sb_t.tile([32, 256], F32, tag="i12z2")
        nc.vector.scalar_tensor_tensor(out=z212[:], in0=Ts[0][:], scalar=1.0,
                                       in1=Ts[3][:], op0=OP.add, op1=OP.mult)
        nc.vector.tensor_tensor_scan(c2_12[:, 0:128], sf12[:, 0:128],
                                     z212[:, 0:128], 0.0, OP.mult, OP.add)
        nc.vector.tensor_tensor_scan(c2_12[:, 128:256], sf12[:, 128:256],
                                     z212[:, 128:256], 0.0, OP.mult, OP.add)
        tc12 = sb_t.tile([32, 256], F32, tag="i12tc")
        nc.scalar.activation(tc12[:], c2_12[:], AF.Tanh, scale=0.5)
        h2x12 = sb_t.tile([32, 256], F32, tag="i12h2x")
        nc.vector.scalar_tensor_tensor(out=h2x12[:], in0=Ts[2][:], scalar=1.0,
                                       in1=tc12[:], op0=OP.add, op1=OP.mult)
        nc.vector.tensor_scalar(h1[:, 1:129], h2x12[:, 0:128], 0.5, None, OP.mult)
        nc.vector.tensor_scalar(h2[0:32, 1:129], h2x12[:, 128:256], 0.5,
                                None, OP.mult)
        for m in range(6 + 2 * it, 6 + 2 * it + 2):
            gemm_tail(m)

    for m in range(6 + 2 * NITER, 6 + 2 * NITER + 6):
        gemm_tail(m)

    # ---------------- finals -> projections -> decoder step 0 ----------------
    hc = sb.tile([52, 2], F32, tag="hc")     # [hfA;hbB;hfE;hbF] | c_flat
    nc.sync.dma_start(hc[0:13, 0:1], h0[0:13, 256:257].bitcast(F32))
    nc.scalar.dma_start(hc[13:26, 0:1], h0[16:29, 256:257].bitcast(F32))
    nc.sync.dma_start(hc[26:39, 0:1], h2[0:13, 128:129].bitcast(F32))
    nc.scalar.dma_start(hc[39:52, 0:1], h2[16:29, 128:129].bitcast(F32))
    nc.sync.dma_start(hc[0:13, 1:2], c2_0[0:13, 255:256])
    nc.scalar.dma_start(hc[13:26, 1:2], c2_0[16:29, 255:256])
    nc.sync.dma_start(hc[26:39, 1:2], c2_12[0:13, 255:256])
    nc.scalar.dma_start(hc[39:52, 1:2], c2_12[16:29, 255:256])
    nc.vector.tensor_scalar(hc[:, 1:2], hc[:, 1:2], 0.5, None, OP.mult)

    scr = ps_ms.tile([128, 512], F32, tag="gz")
    nc.tensor.matmul(scr[0:32, 110:111], p1t[:], hc[:, 0:1],
                     start=True, stop=True)
    nc.tensor.matmul(scr[0:32, 111:112], p2t[:], hc[:, 1:2],
                     start=True, stop=True)
    st = sb.tile([32, 2], F32, tag="st")
    nc.vector.tensor_tensor(out=st[:], in0=scr[0:32, 110:112], in1=pb[:], op=OP.add)
    nc.tensor.matmul(scr[0:128, 120:121], whhd[:], st[:, 0:1],
                     start=True, stop=True)
    Tsd = gate_tanh(scr, 1, slice(120, 121), udh4, "d0")
    z2d = sb.tile([32, 1], F32, tag="z2d")
    nc.vector.scalar_tensor_tensor(out=z2d[:], in0=Tsd[0][:], scalar=1.0,
                                   in1=Tsd[3][:], op0=OP.add, op1=OP.mult)
    fc2d = sb.tile([32, 1], F32, tag="fc2d")
    nc.vector.scalar_tensor_tensor(out=fc2d[:], in0=Tsd[1][:], scalar=1.0,
                                   in1=st[:, 1:2], op0=OP.add, op1=OP.mult)
    c2d = sb.tile([32, 1], F32, tag="c2d")
    nc.vector.tensor_tensor(out=c2d[:], in0=z2d[:], in1=fc2d[:], op=OP.add)
    o0 = sb.tile([33, 1], F32, tag="o0")
    nc.gpsimd.memset(o0[:], 1.0)                 # row 32 stays 1 (bias row)
    cell_tail(Tsd, 1, c2d[:], o0[0:32, :], "d0")

    # out0 becomes dec_h column 0 -> logits row 0 comes out of the main GEMM
    nc.vector.tensor_copy(dh[0][0:32, 0:1], o0[0:32, :])

    # ---------------- main GEMM: leftover m-tiles ----------------
    # leftover tails and heads interleaved to keep PE dense after the encoder
    heads = list(range(32))
    tails = list(range(6 + 2 * NITER + 6, 32))
    while heads or tails:
        if tails:
            gemm_tail(tails.pop(0))
        if heads:
            gemm_head(heads.pop(0))


def _host_prep(inputs):
    x = np.ascontiguousarray(np.asarray(inputs["x"], dtype=np.int32))
    emb = np.ascontiguousarray(np.asarray(inputs["emb"], dtype=np.float32))
    l0, l1 = inputs["enc_params"]
    wih0, whh0, bi0, bh0 = _pack_dir(l0[0], l0[1])
    wih1, whh1, bi1, bh1 = _pack_dir(l1[0], l1[1])
    wihd, whhd, bid_, bhd = _pack_dir(inputs["dec_params"][0], inputs["dec_params"][1])
    out_w = np.asarray(inputs["out_w"], dtype=np.float32)
    out_b = np.asarray(inputs["out_b"], dtype=np.float32)

    fm, bm = _dir_col_mask(0), _dir_col_mask(1)
    wih0f = np.ascontiguousarray((wih0 * fm[:, None]).T)
    wih0b = np.ascontiguousarray((wih0 * bm[:, None]).T)
    wih1_l = np.ascontiguousarray(wih1.T)                # (26, 128)
    wih1_p = np.zeros((32, 128), np.float32)
    wih1_p[0:13] = wih1_l[0:13]                          # yf rows
    wih1_p[16:29] = wih1_l[13:26]                        # yb rows
    yf_rows = np.zeros(32, np.float32); yf_rows[0:13] = 1
    yb_rows = np.zeros(32, np.float32); yb_rows[16:29] = 1
    wih1a = wih1_p * (yf_rows[:, None] * fm[None, :] + yb_rows[:, None] * bm[None, :])
    wih1b = wih1_p * (yb_rows[:, None] * fm[None, :] + yf_rows[:, None] * bm[None, :])

    wb_full = np.zeros((33, 32000), np.float32)
    wb_full[0:13] = out_w.T[0:13]
    wb_full[16:29] = out_w.T[13:26]
    wb_full[32] = out_b

    def proj_pack(w):
        w = np.asarray(w, np.float32)                    # (26, 52)
        p = np.zeros((52, 32), np.float32)
        p[:, 0:13] = w[0:13].T
        p[:, 16:29] = w[13:26].T
        return np.ascontiguousarray(p)

    def bias_pack32(b):
        b = np.asarray(b, np.float32)
        p = np.zeros(32, np.float32)
        p[0:13] = b[0:13]; p[16:29] = b[13:26]
        return p

    shared = {
        "x": x.reshape(1, L),
        "x_win": np.concatenate([x[L - 2 * W:L], x[0:2 * W]]),
        "emb": emb,
        "wih0f": wih0f, "wih0b": wih0b,
        "wih1a": np.ascontiguousarray(wih1a), "wih1b": np.ascontiguousarray(wih1b),
        "whh0": np.ascontiguousarray(whh0), "whh1": np.ascontiguousarray(whh1),
        "whhd": np.ascontiguousarray(whhd),
        "b0": np.stack([bi0, bh0], 1),
        "b1i_r": np.ascontiguousarray(bi1[None, :]),
        "b1h_r": np.ascontiguousarray(bh1[None, :]),
        "bdg": np.ascontiguousarray(
            np.concatenate([_group_cols(bid_), _group_cols(bhd)], 1)),
        "wd_g": np.ascontiguousarray(_group_cols(wihd[:, 0])),
        "wd_r": np.ascontiguousarray(wihd.T),            # (1, 128)
        "p1t": proj_pack(inputs["proj1_w"]), "p2t": proj_pack(inputs["proj2_w"]),
        "pb": np.stack([bias_pack32(inputs["proj1_b"]),
                        bias_pack32(inputs["proj2_b"])], 1),
    }
    per_core = []
    for c in range(NCORE):
        m = dict(shared)
        m["x_my"] = np.ascontiguousarray(x[ESH * c:ESH * (c + 1)])
        m["wb"] = np.ascontiguousarray(wb_full[:, VSH * c:VSH * (c + 1)])
        per_core.append(m)
    return per_core


def _declare(nc):
    A = {}
    spec = {
        "x": ((1, L), I32), "x_win": ((4 * W,), I32), "x_my": ((ESH,), I32),
        "emb": ((32001, 1024), F32),
        "wih0f": ((1024, 128), F32R), "wih0b": ((1024, 128), F32R),
        "wih1a": ((32, 128), F32R), "wih1b": ((32, 128), F32R),
        "whh0": ((32, 128), F32R), "whh1": ((32, 128), F32R), "whhd": ((32, 128), F32),
        "b0": ((128, 2), F32), "b1i_r": ((1, 128), F32), "b1h_r": ((1, 128), F32),
        "bdg": ((32, 8), F32), "wd_g": ((32, 4), F32), "wd_r": ((1, 128), F32),
        "p1t": ((52, 32), F32), "p2t": ((52, 32), F32), "pb": ((32, 2), F32),
        "wb": ((33, VSH), F32R),
    }
    for name, (shape, dt) in spec.items():
        A[name] = nc.dram_tensor(name, list(shape), dt, kind="ExternalInput").ap()
    for name, shape in (("logits_t", (VSH, L)),
                        ("embs_my", (ESH, 1024))):
        A[name] = nc.dram_tensor(name, list(shape), F32, kind="ExternalOutput").ap()
    return A


def build_nc():
    nc = bacc.Bacc("TRN2", target_bir_lowering=False, debug=False,
                   num_devices=NCORE)
    A = _declare(nc)
    with tile.TileContext(nc) as tc:
        with ExitStack() as ctx:
            _emit(ctx, tc, A)
    nc.compile()
    return nc


def kernel(_trace=False, _bench=None, **inputs):
    in_maps = _host_prep(inputs)
    nc = build_nc()
    nc.m = get_hw_module(nc.m)
    res = bass_utils.run_bass_kernel_spmd(
        nc, in_maps, core_ids=list(range(NCORE)), trace=_trace)
    if _bench is not None:
        _bench["exec_time_ns"] = res.exec_time_ns
        _bench["res"] = res
    logits = np.empty((L, 32000), np.float32)
    embs = np.empty((L, 1024), np.float32)
    for c in range(NCORE):
        out = res.results[c]
        logits[:, VSH * c:VSH * (c + 1)] = out["logits_t"].T
        embs[ESH * c:ESH * (c + 1)] = out["embs_my"]
    return logits, embs
